# revision 1
# baseline (speedup 1.0000x reference)
"""AdaptiveTripletLoss on 8 TRN2 NeuronCores.

Device: the compute-dominant Gram matrix G = E @ E^T (4096x4096x2048,
68.7 GFLOP) in bf16 on the PE, f32 PSUM. Symmetry-aware: only the 36
upper-triangular 512x512 blocks are computed, slot-packed 5 per core
(4 cores carry one duplicate pad block). Host mirrors the blocks,
then does masks/counts, order-statistic selection (value-stable under
bf16 distance jitter), exact d_ap/d_an norms and the final masked mean.
"""

import os

import numpy as np
import ml_dtypes

N, D = 4096, 2048
NUM_IDS = 512
N_CORES = 8
MARGIN = 0.3
RATIOS = (0.3, 0.4, 0.3)
EPS = 1e-6

B = 512           # block edge
NB = N // B       # 8x8 block grid
SLOTS = 5         # blocks per core (36 real + 4 pad)
KT = D // 128     # 16 k-tiles

LAST_EXEC_NS = None

_BF16 = ml_dtypes.bfloat16


def _block_assignment():
    """Each core gets 3 blocks sharing row-group A plus 2 sharing row-group
    B (36 real upper-tri blocks + 4 duplicate pads) so the per-core lhs
    input is just two 512-row slices instead of five."""
    c3 = [(0, [0, 1, 2]), (0, [3, 4, 5]), (1, [1, 2, 3]), (2, [2, 3, 4]),
          (2, [5, 6, 7]), (3, [3, 4, 5]), (4, [4, 5, 6]), (5, [5, 6, 7])]
    c2 = [(0, [6, 7]), (1, [4, 5]), (1, [6, 7]), (3, [6, 7]),
          (6, [6, 7]), (4, [7, 7]), (7, [7, 7]), (0, [0, 1])]
    per_core = []
    for (ra, cas), (rb, cbs) in zip(c3, c2):
        per_core.append([(ra, c) for c in cas] + [(rb, c) for c in cbs])
    return per_core


_ASSIGN = _block_assignment()


def _build_gram_kernel():
    import concourse.bacc as bacc
    import concourse.tile as tile
    from concourse import mybir

    nc = bacc.Bacc(None, target_bir_lowering=False)

    f32 = mybir.dt.float32
    bf16 = mybir.dt.bfloat16

    fp8 = mybir.dt.float8e4
    W = SLOTS * B  # 2560 packed rhs columns
    lhsP = nc.declare_dram_parameter("lhsP", [D, 2 * B], fp8, isOutput=False)
    rhsP = nc.declare_dram_parameter("rhsP", [D, W], fp8, isOutput=False)
    out = nc.declare_dram_parameter("out", [W, B], bf16, isOutput=True)

    GRP = (0, 0, 0, 1, 1)  # slot -> lhs row-group
    TT = KT // 2  # 8 DoubleRow steps, each contracting 256 k-rows

    with tile.TileContext(nc) as tc:
        with (
            tc.tile_pool(name="lhs_p", bufs=1) as lhs_pool,
            tc.tile_pool(name="rhs_p", bufs=1) as rhs_pool,
            tc.tile_pool(name="psum", bufs=8, space="PSUM") as psum_pool,
            tc.tile_pool(name="outp", bufs=8) as out_pool,
        ):
            lhs_t, rhs_t = {}, {}

            # Tiles are [128, 2, B] fp8: dim1 holds the adjacent k-tile
            # pair one DoubleRow matmul contracts in a single pass.
            def load_lhs(issuer, g, t):
                tl = lhs_pool.tile([128, 2, B], fp8, tag=f"l{g}_{t}")
                for i in range(2):
                    k = 2 * t + i
                    issuer.dma_start(
                        tl[:, i, :],
                        lhsP[k * 128:(k + 1) * 128, g * B:(g + 1) * B],
                    )
                lhs_t[(g, t)] = tl

            def load_rhs(issuer, s, t):
                tl = rhs_pool.tile([128, 2, B], fp8, tag=f"r{s}_{t}")
                for i in range(2):
                    k = 2 * t + i
                    issuer.dma_start(
                        tl[:, i, :],
                        rhsP[k * 128:(k + 1) * 128, s * B:(s + 1) * B],
                    )
                rhs_t[(s, t)] = tl

            # Critical prefix: slot 0's lhs/rhs pairs go on the two fast
            # HWDGE queues only (~600 ns/issue) so its chain starts
            # streaming ~10 us in; gpsimd's slower SWDGE (~1 us/issue)
            # carries only later-need chunks. Remaining chunks round-robin
            # across all three queues in slot-major (need-by) order.
            for t in range(TT):
                load_lhs(nc.sync, 0, t)
                load_rhs(nc.scalar, 0, t)

            rest = [nc.sync, nc.scalar, nc.gpsimd]
            ri = 0

            def nxt():
                nonlocal ri
                e = rest[ri % 3]
                ri += 1
                return e

            for s in (1, 2):
                for t in range(TT):
                    load_rhs(nxt(), s, t)
            for t in range(TT):
                load_lhs(nxt(), 1, t)
            for s in (3, 4):
                for t in range(TT):
                    load_rhs(nxt(), s, t)

            for s in range(SLOTS):
                for m in range(B // 128):
                    ps = psum_pool.tile([128, B], f32)
                    for t in range(TT):
                        nc.tensor.matmul(
                            ps[:],
                            lhs_t[(GRP[s], t)][:, :, m * 128:(m + 1) * 128],
                            rhs_t[(s, t)][:],
                            start=(t == 0),
                            stop=(t == TT - 1),
                            perf_mode=mybir.MatmulPerfMode.DoubleRow,
                        )
                    ot = out_pool.tile([128, B], bf16)
                    nc.vector.tensor_copy(ot[:], ps[:])
                    r0 = s * B + m * 128
                    # Last slot's outputs drain in parallel across the
                    # by-then-idle queues instead of serializing ~3 us of
                    # transfers into the kernel tail on scalar alone.
                    if s == SLOTS - 1:
                        oeng = [nc.sync, nc.gpsimd, nc.sync, nc.gpsimd][m]
                    else:
                        oeng = nc.scalar
                    oeng.dma_start(out[r0:r0 + 128, :], ot[:])

    nc.compile()
    return nc


_NC_CACHE = None


def _run_gram(emb: np.ndarray) -> np.ndarray:
    """Run the 8-core symmetric Gram kernel; returns G = emb @ emb.T f32."""
    global _NC_CACHE, LAST_EXEC_NS
    from concourse.bass_utils import run_bass_kernel_spmd

    if _NC_CACHE is None:
        _NC_CACHE = _build_gram_kernel()
    nc = _NC_CACHE

    eT_bf = np.ascontiguousarray(emb.T).astype(ml_dtypes.float8_e4m3)
    in_maps = []
    for core in range(N_CORES):
        slots = _ASSIGN[core]
        ra, rb = slots[0][0], slots[3][0]
        lhs = np.concatenate(
            [eT_bf[:, ra * B:(ra + 1) * B], eT_bf[:, rb * B:(rb + 1) * B]],
            axis=1,
        )
        rhs = np.concatenate(
            [eT_bf[:, c * B:(c + 1) * B] for (r, c) in slots], axis=1
        )
        in_maps.append(
            {"lhsP": np.ascontiguousarray(lhs), "rhsP": np.ascontiguousarray(rhs)}
        )

    trace = bool(int(os.environ.get("KERNEL_TRACE", "0")))
    res = run_bass_kernel_spmd(
        nc, in_maps, core_ids=list(range(N_CORES)), trace=trace
    )
    if res.exec_time_ns is not None:
        LAST_EXEC_NS = res.exec_time_ns

    G = np.empty((N, N), dtype=np.float32)
    for core in range(N_CORES):
        o = np.asarray(res.results[core]["out"], dtype=np.float32)
        for s, (r, c) in enumerate(_ASSIGN[core]):
            blk = o[s * B:(s + 1) * B, :]
            G[r * B:(r + 1) * B, c * B:(c + 1) * B] = blk
            if r != c:
                G[c * B:(c + 1) * B, r * B:(r + 1) * B] = blk.T
    return G


def _sample_js(counts: np.ndarray, us: list) -> np.ndarray:
    """Replicate the reference's f32 sampling math. counts [N] int, us 3x[N]
    f32 uniforms. Returns j ranks [N, 3] int64 (rank into the masked sort)."""
    out = []
    for t, r in enumerate(RATIOS):
        cnt = np.maximum(
            np.int32(1),
            np.floor(counts.astype(np.float32) * np.float32(r)).astype(np.int32),
        )
        j = np.minimum((us[t] * cnt.astype(np.float32)).astype(np.int32), cnt - 1)
        out.append(j.astype(np.int64))
    return np.stack(out, axis=1)


def kernel(embeddings: np.ndarray, labels: np.ndarray) -> np.ndarray:
    emb = np.ascontiguousarray(np.asarray(embeddings, dtype=np.float32))
    lab = np.asarray(labels).astype(np.int64)

    G = _run_gram(emb)

    # Selection keys: within row i, ordering by (sq_j - 2 G[i,j]) equals
    # ordering by distance.
    sq = np.einsum("ij,ij->i", emb, emb).astype(np.float32)

    # Uniforms must match jax.random with key 42 bit-exactly.
    import jax

    with jax.default_device(jax.devices("cpu")[0]):
        skey = jax.random.key(42)
        keys = jax.random.split(skey, 6)
        us = [np.asarray(jax.random.uniform(k, (N,))) for k in keys]

    class_size = np.bincount(lab, minlength=NUM_IDS)
    pos_count = class_size[lab] - 1
    neg_count = N - class_size[lab]
    valid = (pos_count > 0) & (neg_count > 0)

    pos_js = _sample_js(pos_count, us[0:3])  # [N, 3]
    neg_js = _sample_js(neg_count, us[3:6])  # [N, 3]

    # Per-class member lists
    order = np.argsort(lab, kind="stable")
    sorted_lab = lab[order]
    starts = np.searchsorted(sorted_lab, np.arange(NUM_IDS), side="left")
    ends = np.searchsorted(sorted_lab, np.arange(NUM_IDS), side="right")

    pos_idx = np.zeros((N, 3), dtype=np.int64)
    neg_idx = np.zeros((N, 3), dtype=np.int64)
    INF = np.float32(np.inf)

    for i in range(N):
        li = lab[i]
        members = order[starts[li]:ends[li]]
        key_row = sq - 2.0 * G[i]  # f32 [N]
        if valid[i]:
            pos_members = members[members != i]
            pk = key_row[pos_members]
            po = np.argsort(pk, kind="stable")
            pos_idx[i] = pos_members[po[pos_js[i]]]
        # negatives: mask out own class and self
        nk = key_row.copy()
        nk[members] = INF
        nk[i] = INF
        kth = np.unique(neg_js[i])
        part = np.argpartition(nk, kth)
        neg_idx[i] = part[neg_js[i]]

    a = emb[:, None, :]
    p = emb[pos_idx]
    ng = emb[neg_idx]
    d_ap = np.sqrt(np.sum((a - p + np.float32(EPS)) ** 2, axis=-1))
    d_an = np.sqrt(np.sum((a - ng + np.float32(EPS)) ** 2, axis=-1))
    tri = np.maximum(d_ap - d_an + np.float32(MARGIN), np.float32(0.0))
    w = valid[:, None].astype(np.float32)
    denom = max(3.0 * float(valid.sum()), 1.0)
    loss = np.float32(np.sum(tri * w) / denom)
    return np.array(loss, dtype=np.float32)



# revision 5
# speedup vs baseline: 1.1473x; 1.1473x over previous
"""AdaptiveTripletLoss on 8 TRN2 NeuronCores.

Device: the compute-dominant Gram matrix G = E @ E^T in fp8 DoubleRow on
the PE. Symmetry-aware: only the 36 upper-triangular 512x512 blocks are
computed. Each core loads 5 packed column-groups (5 MB, one DMA per
512 KB half-group) and runs a fixed 5-cell schedule:
  (s0,s2) (s1,s2) (s0,s3) (s1,s3) (s4,s4)
where s0..s4 are per-core group slots chosen by the host so the union of
all cores' cells covers the 36 blocks (4 duplicates). Outputs are one
512x512 bf16 DMA per cell. Host mirrors blocks, then does masks/counts,
order-statistic selection, exact d_ap/d_an norms and the masked mean.
"""

import os

import numpy as np
import ml_dtypes

N, D = 4096, 2048
NUM_IDS = 512
N_CORES = 8
MARGIN = 0.3
RATIOS = (0.3, 0.4, 0.3)
EPS = 1e-6

B = 512           # block edge
KT = D // 128     # 16 k-tiles per group
TT = KT // 2      # 8 DoubleRow steps
NSLOT = 5
NCELL = 5

LAST_EXEC_NS = None

# Per-core slots (5 group ids each) and the fixed cell schedule over slots.
# Cells: (lhs_slot, rhs_slot) = (0,2),(1,2),(0,3),(1,3),(4,4).
_SLOTS = [
    (0, 1, 2, 3, 4),  # cells 02 03 12 13 + loop 44
    (0, 1, 4, 5, 6),  # 04 05 14 15 + 66
    (2, 3, 6, 7, 5),  # 26 27 36 37 + 55
    (2, 3, 4, 5, 7),  # 24 25 34 35 + 77
    (4, 5, 6, 7, 0),  # 46 47 56 57 + 00
    (0, 1, 6, 7, 2),  # 06 07 16 17 + 22
    (0, 4, 1, 5, 1),  # 01 05* 41* 45 + 11   (*dup)
    (2, 6, 3, 7, 3),  # 23 27* 63* 67 + 33   (*dup)
]
_CELLS = ((0, 2), (1, 2), (0, 3), (1, 3), (4, 4))

_BF16 = ml_dtypes.bfloat16


def _build_gram_kernel():
    import concourse.bacc as bacc
    import concourse.tile as tile
    from concourse import mybir

    nc = bacc.Bacc(None, target_bir_lowering=False)

    f32 = mybir.dt.float32
    bf16 = mybir.dt.bfloat16
    fp8 = mybir.dt.float8e4

    grps = nc.declare_dram_parameter("grps", [NSLOT, 128, KT, B], fp8,
                                     isOutput=False)
    out = nc.declare_dram_parameter("out", [NCELL, 128, 4, B], bf16,
                                    isOutput=True)

    with tile.TileContext(nc) as tc:
        with (
            tc.tile_pool(name="grp_p", bufs=1) as grp_pool,
            tc.tile_pool(name="psum", bufs=8, space="PSUM") as psum_pool,
            tc.tile_pool(name="outp", bufs=8) as out_pool,
        ):
            gt = []
            for s in range(NSLOT):
                gt.append(grp_pool.tile([128, KT, B], fp8, tag=f"g{s}",
                                        name=f"g{s}"))

            # Input DMAs, half-group (512 KB, 4 KB/partition) granularity,
            # ordered by first use: s0,s2 feed cell 0; then s1, s3, s4.
            for s, c in ((0, 0), (2, 0), (0, 1), (2, 1), (1, 0), (1, 1),
                         (3, 0), (3, 1), (4, 0), (4, 1)):
                k0, k1 = c * (KT // 2), (c + 1) * (KT // 2)
                nc.sync.dma_start(gt[s][:, k0:k1, :], grps[s, :, k0:k1, :])

            # Cells in schedule order; within a cell interleave the 4 m-chains
            # across t so the first chunk of data unlocks 16 matmuls before
            # the second chunk is needed.
            for ci, (ls, rs) in enumerate(_CELLS):
                ps = [psum_pool.tile([128, B], f32, name="ps")
                      for m in range(4)]
                for t in range(TT):
                    for m in range(4):
                        nc.tensor.matmul(
                            ps[m][:],
                            gt[ls][:, 2 * t:2 * t + 2, m * 128:(m + 1) * 128],
                            gt[rs][:, 2 * t:2 * t + 2, :],
                            start=(t == 0),
                            stop=(t == TT - 1),
                            perf_mode=mybir.MatmulPerfMode.DoubleRow,
                        )
                ot = out_pool.tile([128, 4, B], bf16, tag=f"o{ci}")
                for m in range(4):
                    nc.vector.tensor_copy(ot[:, m, :], ps[m][:])
                nc.scalar.dma_start(out[ci], ot[:])

    nc.compile()
    return nc


_NC_CACHE = None


def _pack_group(eT_fp8: np.ndarray, g: int) -> np.ndarray:
    """eT [D, N] fp8 -> [128, KT, B] packed column group g."""
    blk = eT_fp8[:, g * B:(g + 1) * B]              # [2048, 512]
    return np.ascontiguousarray(
        blk.reshape(KT, 128, B).transpose(1, 0, 2))  # [128, 16, 512]


def _run_gram(emb: np.ndarray) -> np.ndarray:
    """Run the 8-core symmetric Gram kernel; returns G = emb @ emb.T f32."""
    global _NC_CACHE, LAST_EXEC_NS
    from concourse.bass_utils import run_bass_kernel_spmd

    if _NC_CACHE is None:
        _NC_CACHE = _build_gram_kernel()
    nc = _NC_CACHE

    eT_fp8 = np.ascontiguousarray(emb.T).astype(ml_dtypes.float8_e4m3)
    packed = {g: _pack_group(eT_fp8, g) for g in range(8)}
    in_maps = []
    for core in range(N_CORES):
        arr = np.stack([packed[g] for g in _SLOTS[core]], axis=0)
        in_maps.append({"grps": np.ascontiguousarray(arr)})

    trace = bool(int(os.environ.get("KERNEL_TRACE", "0")))
    res = run_bass_kernel_spmd(
        nc, in_maps, core_ids=list(range(N_CORES)), trace=trace
    )
    if res.exec_time_ns is not None:
        LAST_EXEC_NS = res.exec_time_ns

    G = np.empty((N, N), dtype=np.float32)
    for core in range(N_CORES):
        o = np.asarray(res.results[core]["out"], dtype=np.float32)
        # [NCELL, 128, 4, B] -> [NCELL, 512, B]
        o = o.transpose(0, 2, 1, 3).reshape(NCELL, B, B)
        slots = _SLOTS[core]
        for ci, (ls, rs) in enumerate(_CELLS):
            r, c = slots[ls], slots[rs]
            if r > c:
                continue  # dup of another core's cell (transposed); skip
            blk = o[ci]
            G[r * B:(r + 1) * B, c * B:(c + 1) * B] = blk
            if r != c:
                G[c * B:(c + 1) * B, r * B:(r + 1) * B] = blk.T
    return G


def _sample_js(counts: np.ndarray, us: list) -> np.ndarray:
    """Replicate the reference's f32 sampling math. counts [N] int, us 3x[N]
    f32 uniforms. Returns j ranks [N, 3] int64 (rank into the masked sort)."""
    out = []
    for t, r in enumerate(RATIOS):
        cnt = np.maximum(
            np.int32(1),
            np.floor(counts.astype(np.float32) * np.float32(r)).astype(np.int32),
        )
        j = np.minimum((us[t] * cnt.astype(np.float32)).astype(np.int32), cnt - 1)
        out.append(j.astype(np.int64))
    return np.stack(out, axis=1)


def kernel(embeddings: np.ndarray, labels: np.ndarray) -> np.ndarray:
    emb = np.ascontiguousarray(np.asarray(embeddings, dtype=np.float32))
    lab = np.asarray(labels).astype(np.int64)

    G = _run_gram(emb)

    # Selection keys: within row i, ordering by (sq_j - 2 G[i,j]) equals
    # ordering by distance.
    sq = np.einsum("ij,ij->i", emb, emb).astype(np.float32)

    # Uniforms must match jax.random with key 42 bit-exactly.
    import jax

    with jax.default_device(jax.devices("cpu")[0]):
        skey = jax.random.key(42)
        keys = jax.random.split(skey, 6)
        us = [np.asarray(jax.random.uniform(k, (N,))) for k in keys]

    class_size = np.bincount(lab, minlength=NUM_IDS)
    pos_count = class_size[lab] - 1
    neg_count = N - class_size[lab]
    valid = (pos_count > 0) & (neg_count > 0)

    pos_js = _sample_js(pos_count, us[0:3])  # [N, 3]
    neg_js = _sample_js(neg_count, us[3:6])  # [N, 3]

    # Per-class member lists
    order = np.argsort(lab, kind="stable")
    sorted_lab = lab[order]
    starts = np.searchsorted(sorted_lab, np.arange(NUM_IDS), side="left")
    ends = np.searchsorted(sorted_lab, np.arange(NUM_IDS), side="right")

    pos_idx = np.zeros((N, 3), dtype=np.int64)
    neg_idx = np.zeros((N, 3), dtype=np.int64)
    INF = np.float32(np.inf)

    for i in range(N):
        li = lab[i]
        members = order[starts[li]:ends[li]]
        key_row = sq - 2.0 * G[i]  # f32 [N]
        if valid[i]:
            pos_members = members[members != i]
            pk = key_row[pos_members]
            po = np.argsort(pk, kind="stable")
            pos_idx[i] = pos_members[po[pos_js[i]]]
        # negatives: mask out own class and self
        nk = key_row.copy()
        nk[members] = INF
        nk[i] = INF
        kth = np.unique(neg_js[i])
        part = np.argpartition(nk, kth)
        neg_idx[i] = part[neg_js[i]]

    a = emb[:, None, :]
    p = emb[pos_idx]
    ng = emb[neg_idx]
    d_ap = np.sqrt(np.sum((a - p + np.float32(EPS)) ** 2, axis=-1))
    d_an = np.sqrt(np.sum((a - ng + np.float32(EPS)) ** 2, axis=-1))
    tri = np.maximum(d_ap - d_an + np.float32(MARGIN), np.float32(0.0))
    w = valid[:, None].astype(np.float32)
    denom = max(3.0 * float(valid.sum()), 1.0)
    loss = np.float32(np.sum(tri * w) / denom)
    return np.array(loss, dtype=np.float32)


# revision 6
# speedup vs baseline: 1.2008x; 1.0467x over previous
"""AdaptiveTripletLoss on 8 TRN2 NeuronCores.

Device: the compute-dominant Gram matrix G = E @ E^T in fp8 DoubleRow on
the PE, symmetry-aware (upper-triangular blocks only). Generic chain
machinery: each core loads NSLOT packed 512-column groups (each a pair
of 256-row half-groups, 4 chunks of 4 k-tiles per slot for fine-grained
DMA/compute overlap) and runs a fixed shared CHAINS schedule; the host
picks per-core slot contents so the union covers all of upper(G).
Dummy warm-up matmuls un-throttle the PE clock (HAM) while input DMAs
stream. Host mirrors blocks, then does masks/counts, order-statistic
selection, exact d_ap/d_an norms and the masked mean.
"""

import os

import numpy as np
import ml_dtypes

N, D = 4096, 2048
NUM_IDS = 512
N_CORES = 8
MARGIN = 0.3
RATIOS = (0.3, 0.4, 0.3)
EPS = 1e-6

B = 512           # block edge / slot width
HALF = 256        # half-group rows
KT = D // 128     # 16 k-tiles per slot
NCHUNK = 4        # 4 k-tiles per chunk
TT = KT // 2      # 8 DoubleRow steps per chain
N_WARM = 9        # dummy warm-up matmuls (~3.8 us cold => HAM warm)

LAST_EXEC_NS = None

# ---- cover definition (v2: 5 full-group slots, 20 chains) ----
_GROUP_SLOTS = [
    (0, 1, 2, 3, 4), (0, 1, 4, 5, 6), (2, 3, 6, 7, 5), (2, 3, 4, 5, 7),
    (4, 5, 6, 7, 0), (0, 1, 6, 7, 2), (0, 4, 1, 5, 1), (2, 6, 3, 7, 3),
]
_CELLS = ((0, 2), (1, 2), (0, 3), (1, 3), (4, 4))

NSLOT = 5
# SLOTPACK[core][slot] = (half-group, half-group): rows h*256..h*256+255
SLOTPACK = [[(2 * g, 2 * g + 1) for g in gs] for gs in _GROUP_SLOTS]
# shared schedule: chain = (lhs_slot, m, rhs_slot); grouped per 4 for
# psum interleaving
CHAINS = [(ls, m, rs) for (ls, rs) in _CELLS for m in range(4)]
CHAIN_GROUPS = [list(range(i, min(i + 4, len(CHAINS))))
                for i in range(0, len(CHAINS), 4)]
NCHAIN = len(CHAINS)


def _dma_order():
    """Input chunk order: first chain-group's slots chunk-interleaved,
    then remaining slots in first-use order."""
    first = []
    for ci in CHAIN_GROUPS[0]:
        ls, _, rs = CHAINS[ci]
        for s in (ls, rs):
            if s not in first:
                first.append(s)
    rest = []
    for (ls, _, rs) in CHAINS:
        for s in (ls, rs):
            if s not in first and s not in rest:
                rest.append(s)
    order = []
    for c in range(NCHUNK):
        for s in first:
            order.append((s, c))
    for s in rest:
        for c in range(NCHUNK):
            order.append((s, c))
    return order


def _build_gram_kernel():
    import concourse.bacc as bacc
    import concourse.tile as tile
    from concourse import mybir

    nc = bacc.Bacc(None, target_bir_lowering=False)

    f32 = mybir.dt.float32
    bf16 = mybir.dt.bfloat16
    fp8 = mybir.dt.float8e4

    grps = nc.declare_dram_parameter("grps", [NSLOT, 128, KT, B], fp8,
                                     isOutput=False)
    out = nc.declare_dram_parameter("out", [NCHAIN, 128, B], bf16,
                                    isOutput=True)

    with tile.TileContext(nc) as tc:
        with (
            tc.tile_pool(name="grp_p", bufs=1) as grp_pool,
            tc.tile_pool(name="psum", bufs=8, space="PSUM") as psum_pool,
            tc.tile_pool(name="outp", bufs=6) as out_pool,
        ):
            gch = [[grp_pool.tile([128, NCHUNK, B], fp8, name=f"g{s}_{c}")
                    for c in range(NCHUNK)] for s in range(NSLOT)]
            dmy = grp_pool.tile([128, 2, B], fp8, name="dmy")

            for s, c in _dma_order():
                k0 = c * NCHUNK
                nc.sync.dma_start(gch[s][c][:], grps[s, :, k0:k0 + NCHUNK, :])

            # PE warm-up on a zeroed tile while inputs stream.
            nc.gpsimd.memset(dmy[:], 0.0)
            for i in range(N_WARM):
                wp = psum_pool.tile([128, B], f32, name="ps")
                nc.tensor.matmul(
                    wp[:], dmy[:, :, 0:128], dmy[:],
                    start=True, stop=True,
                    perf_mode=mybir.MatmulPerfMode.DoubleRow,
                )

            for grp in CHAIN_GROUPS:
                pss = [psum_pool.tile([128, B], f32, name="ps") for _ in grp]
                for t in range(TT):
                    ct = t // 2
                    o = 2 * (t % 2)
                    for j, ci in enumerate(grp):
                        ls, m, rs = CHAINS[ci]
                        nc.tensor.matmul(
                            pss[j][:],
                            gch[ls][ct][:, o:o + 2, m * 128:(m + 1) * 128],
                            gch[rs][ct][:, o:o + 2, :],
                            start=(t == 0),
                            stop=(t == TT - 1),
                            perf_mode=mybir.MatmulPerfMode.DoubleRow,
                        )
                for j, ci in enumerate(grp):
                    ot = out_pool.tile([128, B], bf16, name="ot")
                    nc.vector.tensor_copy(ot[:], pss[j][:])
                    nc.scalar.dma_start(out[ci], ot[:])

    nc.compile()
    return nc


_NC_CACHE = None


def _pack_slot(eT8: np.ndarray, pair) -> np.ndarray:
    """eT8 [D, N] fp8 -> [128, KT, B] packed slot of two half-groups."""
    h0, h1 = pair
    blk = np.concatenate(
        [eT8[:, h0 * HALF:(h0 + 1) * HALF], eT8[:, h1 * HALF:(h1 + 1) * HALF]],
        axis=1)                                      # [2048, 512]
    return np.ascontiguousarray(
        blk.reshape(KT, 128, B).transpose(1, 0, 2))  # [128, 16, 512]


def _run_gram(emb: np.ndarray) -> np.ndarray:
    """Run the 8-core symmetric Gram kernel; returns G = emb @ emb.T f32."""
    global _NC_CACHE, LAST_EXEC_NS
    from concourse.bass_utils import run_bass_kernel_spmd

    if _NC_CACHE is None:
        _NC_CACHE = _build_gram_kernel()
    nc = _NC_CACHE

    eT8 = np.ascontiguousarray(emb.T).astype(ml_dtypes.float8_e4m3)
    pack_cache = {}
    in_maps = []
    for core in range(N_CORES):
        slabs = []
        for pair in SLOTPACK[core]:
            if pair not in pack_cache:
                pack_cache[pair] = _pack_slot(eT8, pair)
            slabs.append(pack_cache[pair])
        in_maps.append({"grps": np.ascontiguousarray(np.stack(slabs, axis=0))})

    trace = bool(int(os.environ.get("KERNEL_TRACE", "0")))
    res = run_bass_kernel_spmd(
        nc, in_maps, core_ids=list(range(N_CORES)), trace=trace
    )
    if res.exec_time_ns is not None:
        LAST_EXEC_NS = res.exec_time_ns

    G = np.empty((N, N), dtype=np.float32)
    for core in range(N_CORES):
        o = np.asarray(res.results[core]["out"], dtype=np.float32)  # [NCHAIN,128,B]
        S = SLOTPACK[core]
        for ci, (ls, m, rs) in enumerate(CHAINS):
            r0 = S[ls][m // 2] * HALF + (m % 2) * 128
            strip = o[ci]                       # [128, 512]
            for half in range(2):
                c0 = S[rs][half] * HALF
                piece = strip[:, half * HALF:(half + 1) * HALF]  # [128, 256]
                G[r0:r0 + 128, c0:c0 + HALF] = piece
                G[c0:c0 + HALF, r0:r0 + 128] = piece.T
    return G


def _sample_js(counts: np.ndarray, us: list) -> np.ndarray:
    """Replicate the reference's f32 sampling math. counts [N] int, us 3x[N]
    f32 uniforms. Returns j ranks [N, 3] int64 (rank into the masked sort)."""
    out = []
    for t, r in enumerate(RATIOS):
        cnt = np.maximum(
            np.int32(1),
            np.floor(counts.astype(np.float32) * np.float32(r)).astype(np.int32),
        )
        j = np.minimum((us[t] * cnt.astype(np.float32)).astype(np.int32), cnt - 1)
        out.append(j.astype(np.int64))
    return np.stack(out, axis=1)


def kernel(embeddings: np.ndarray, labels: np.ndarray) -> np.ndarray:
    emb = np.ascontiguousarray(np.asarray(embeddings, dtype=np.float32))
    lab = np.asarray(labels).astype(np.int64)

    G = _run_gram(emb)

    # Selection keys: within row i, ordering by (sq_j - 2 G[i,j]) equals
    # ordering by distance.
    sq = np.einsum("ij,ij->i", emb, emb).astype(np.float32)

    # Uniforms must match jax.random with key 42 bit-exactly.
    import jax

    with jax.default_device(jax.devices("cpu")[0]):
        skey = jax.random.key(42)
        keys = jax.random.split(skey, 6)
        us = [np.asarray(jax.random.uniform(k, (N,))) for k in keys]

    class_size = np.bincount(lab, minlength=NUM_IDS)
    pos_count = class_size[lab] - 1
    neg_count = N - class_size[lab]
    valid = (pos_count > 0) & (neg_count > 0)

    pos_js = _sample_js(pos_count, us[0:3])  # [N, 3]
    neg_js = _sample_js(neg_count, us[3:6])  # [N, 3]

    # Per-class member lists
    order = np.argsort(lab, kind="stable")
    sorted_lab = lab[order]
    starts = np.searchsorted(sorted_lab, np.arange(NUM_IDS), side="left")
    ends = np.searchsorted(sorted_lab, np.arange(NUM_IDS), side="right")

    pos_idx = np.zeros((N, 3), dtype=np.int64)
    neg_idx = np.zeros((N, 3), dtype=np.int64)
    INF = np.float32(np.inf)

    for i in range(N):
        li = lab[i]
        members = order[starts[li]:ends[li]]
        key_row = sq - 2.0 * G[i]  # f32 [N]
        if valid[i]:
            pos_members = members[members != i]
            pk = key_row[pos_members]
            po = np.argsort(pk, kind="stable")
            pos_idx[i] = pos_members[po[pos_js[i]]]
        # negatives: mask out own class and self
        nk = key_row.copy()
        nk[members] = INF
        nk[i] = INF
        kth = np.unique(neg_js[i])
        part = np.argpartition(nk, kth)
        neg_idx[i] = part[neg_js[i]]

    a = emb[:, None, :]
    p = emb[pos_idx]
    ng = emb[neg_idx]
    d_ap = np.sqrt(np.sum((a - p + np.float32(EPS)) ** 2, axis=-1))
    d_an = np.sqrt(np.sum((a - ng + np.float32(EPS)) ** 2, axis=-1))
    tri = np.maximum(d_ap - d_an + np.float32(MARGIN), np.float32(0.0))
    w = valid[:, None].astype(np.float32)
    denom = max(3.0 * float(valid.sum()), 1.0)
    loss = np.float32(np.sum(tri * w) / denom)
    return np.array(loss, dtype=np.float32)
